# revision 24
# baseline (speedup 1.0000x reference)
"""Trainium2 Bass kernel for nn_Attention_27977416966318 (sparse_attention).

score[b,s] = v . tanh(W @ concat(static[b,s], dynamic[b,s], dec[b]))
out = softmax(score, axis=1)

Shapes: static/dynamic [64, 2048, 256] f32, decoder_hidden [64, 256],
v [1, 768], W [768, 768].  Output [64, 2048] f32.

Strategy: data-parallel over batch B=64 across 8 NeuronCores (8 batches
per core).  W @ cat = W12 @ [static;dynamic] + (W3 @ dec[b]), the last
term a per-batch bias computed once on-device in bf16.  The main GEMM
(contraction 512, outputs 768, 16384 tokens/core) runs in fp8-e4m3 with
perf_mode=DoubleRow (2 fp8 MACs/cell/cycle): W scaled x256, x scaled
x16, PSUM = 4096*h; tanh activation applies scale=1/4096 + bias.

fp8 precision rescue: a first-order mean-field correction
  c[t] = gamma * [ (dW^T v) . x~_t + (W~^T v) . dx_t ],  gamma ~ E[tanh']
computed on the HOST (rank-2 dot per token, O(T*k)) and added to the
scores on-device (DVE add into the score PSUM) before exp.  This cuts
max rel err from ~2.6e-2 (pure fp8) to ~1.2e-2 vs the 2e-2 gate.

The v-dot runs as 4 column-packed (tile_position) M=1 bf16 matmuls on
disjoint 32-col strips.  exp fused into score PSUM->SBUF copy with
accum_out giving per-chunk softmax denominators; final reduce+scale per
core on an [8, 2048] tile.
"""

import os

import numpy as np
import ml_dtypes

import concourse.bass as bass
from concourse import bacc
import concourse.mybir as mybir
import concourse.tile as tile
from concourse.bass_utils import run_bass_kernel_spmd

B, S, H = 64, 2048, 256
H3 = 3 * H          # 768
NCORES = 8
BL = B // NCORES    # 8 batches per core
T = BL * S          # 16384 tokens per core
KP = 2              # DoubleRow k-pair tiles (2 x 256 = 512 contraction)
MT = H3 // 128      # 6 output o-tiles
GT = 1024           # tokens per group (2 chunks of 512)
F32 = mybir.dt.float32
BF16 = mybir.dt.bfloat16
FP8 = mybir.dt.float8e4
DR = mybir.MatmulPerfMode.DoubleRow
TANH = mybir.ActivationFunctionType.Tanh
EXP = mybir.ActivationFunctionType.Exp

SW = 256.0          # W fp8 scale
SX = 16.0           # x fp8 scale
GAMMA = 0.5464      # E[tanh'(h)] mean-field coefficient

_CACHED = {}


def build_bass():
    nc = bacc.Bacc(None, target_bir_lowering=False, debug=False)
    x = nc.dram_tensor("x_t", [2 * H, T], FP8, kind="ExternalInput")
    xr = x.rearrange("(t p) n -> p t n", p=128)  # [128, 4, T]
    dec = nc.dram_tensor("dec_t", [H, BL], BF16, kind="ExternalInput")
    wt = nc.dram_tensor("wt", [2 * H, H3], FP8, kind="ExternalInput")
    wt3 = nc.dram_tensor("wt3", [H, H3], BF16, kind="ExternalInput")
    vv = nc.dram_tensor("v", [1, H3], F32, kind="ExternalInput")
    corr = nc.dram_tensor("corr", [BL, S], BF16, kind="ExternalInput")
    eye = nc.dram_tensor("eye", [4 * BL, 4 * BL], BF16, kind="ExternalInput")
    out = nc.dram_tensor("out", [BL, S], F32, kind="ExternalOutput")

    with tile.TileContext(nc) as tc:
        with (
            tc.tile_pool(name="const", bufs=1) as constp,
            tc.tile_pool(name="xp", bufs=2) as xp,
            tc.tile_pool(name="thp", bufs=13) as thp,
            tc.tile_pool(name="misc", bufs=1) as miscp,
            tc.tile_pool(name="hps", bufs=3, space="PSUM") as hps,
            tc.tile_pool(name="sps", bufs=2, space="PSUM") as sps,
        ):
            # ---- PE warmup: zero matmuls so the HAM clock gate reaches
            # 8/8 and stays there by the time the real stream begins ----
            warm = constp.tile([128, 512], BF16)
            nc.vector.memset(warm, 0.0)
            warm_ps = sps.tile([128, 512], F32, tag="s", name="warm_ps")
            for i in range(16):
                nc.tensor.matmul(
                    out=warm_ps, lhsT=warm[:, 0:128], rhs=warm,
                    start=True, stop=True,
                )

            # ---- first x tiles: spread the first two groups' k-tiles
            # across 4 queues so the first matmuls can start ASAP ----
            first_eng = [nc.sync, nc.scalar, nc.gpsimd, nc.sync]
            first_xt = xp.tile([128, 2 * KP, GT], FP8, tag="x", bufs=3, name="x_0_0")
            for kt_i in range(2 * KP):
                first_eng[kt_i].dma_start(
                    out=first_xt[:, kt_i, :], in_=xr[:, kt_i, 0:GT]
                )
            second_xt = xp.tile([128, 2 * KP, GT], FP8, tag="x", bufs=3, name="x_0_1")
            for kt_i in range(2 * KP):
                first_eng[kt_i].dma_start(
                    out=second_xt[:, kt_i, :], in_=xr[:, kt_i, GT : 2 * GT]
                )

            # ---- constants on the scalar (HWDGE) queue, in parallel ----
            # wt is W12.T: [k, o] fp8 x256.  k-pair tiles (0,1) and (2,3).
            wtr = wt.rearrange("(t p) o -> p t o", p=128)
            wt_sb = constp.tile([128, 2 * KP, H3], FP8)
            nc.scalar.dma_start(out=wt_sb[:, 0:2, :], in_=wtr[:, 0:2, :])
            nc.scalar.dma_start(out=wt_sb[:, 2:4, :], in_=wtr[:, 2:4, :])
            # wt3 is W3.T: [k, o] bf16 for the decoder bias matmul
            wt3r = wt3.rearrange("(t p) o -> p t o", p=128)
            wt3_sb = constp.tile([128, 2, H3], BF16)
            nc.scalar.dma_start(out=wt3_sb, in_=wt3r)
            # v in bf16: the v-dot runs as a bf16 matmul (col-packable)
            v_sb = constp.tile([128, MT], BF16)
            nc.gpsimd.dma_start(out=v_sb, in_=vv[0].rearrange("(t p) -> p t", p=128))
            dec_sb = constp.tile([128, 2, BL], BF16)
            nc.gpsimd.dma_start(
                out=dec_sb, in_=dec.rearrange("(t p) b -> p t b", p=128)
            )
            # host-side mean-field correction c, [4*BL, 512] bf16; row 4b+ci
            # is chunk ci of batch b.  Injected into the score accumulation
            # as a K=1 matmul (lhsT = 1.0 at the same partition).
            c_sb = constp.tile([4 * BL, 512], BF16)
            nc.gpsimd.dma_start(out=c_sb, in_=corr[:, :].rearrange(
                "b (c f) -> (b c) f", c=4))
            eye_sb = constp.tile([4 * BL, 4 * BL], BF16)
            nc.gpsimd.dma_start(out=eye_sb, in_=eye[:, :])

            bias_sb = constp.tile([128, MT, BL], F32)

            def emit_bias():
                # bias[o, b] = sum_k W3T[k, o] dec[k, b]   (bf16)
                for m in range(MT):
                    bias_ps = sps.tile([128, BL], F32, tag="s", name=f"bias_ps_{m}")
                    for i in range(2):
                        nc.tensor.matmul(
                            out=bias_ps,
                            lhsT=wt3_sb[:, i, m * 128 : (m + 1) * 128],
                            rhs=dec_sb[:, i, :],
                            start=(i == 0),
                            stop=(i == 1),
                        )
                    nc.vector.tensor_copy(out=bias_sb[:, m, :], in_=bias_ps)

            # per b: 4 chunks x (512 exp-scores + 1 chunk-sum)
            escomb = miscp.tile([BL, 4, 513], F32)
            esum = miscp.tile([BL, 1], F32)

            # ---- main loop ----
            for b in range(BL):
                ths = {}
                # one PSUM bank holds the 4 chunk scores on partitions
                # 0/32/64/96 (column-group packing); the other rows are
                # zeroed (early, off the critical path) so one full-height
                # exp can read the whole bank
                score_ps = sps.tile([128, 512], F32, tag="s", name=f"sa_{b}")
                nc.vector.memset(score_ps, 0.0)
                for g in range(2):
                    tok0 = b * S + g * GT
                    if b == 0 and g == 0:
                        xt = first_xt
                    elif b == 0 and g == 1:
                        xt = second_xt
                    else:
                        xt = xp.tile(
                            [128, 2 * KP, GT], FP8, tag="x", bufs=3, name=f"x_{b}_{g}"
                        )
                        nc.sync.dma_start(
                            out=xt, in_=xr[:, :, tok0 : tok0 + GT]
                        )
                    for m in range(MT):
                        h_ps = hps.tile([128, GT], F32, tag="h", name=f"h_{b}_{g}_{m}")
                        for j in range(KP):
                            for c in range(2):
                                nc.tensor.matmul(
                                    out=h_ps[:, c * 512 : (c + 1) * 512],
                                    lhsT=wt_sb[:, 2 * j : 2 * j + 2,
                                               m * 128 : (m + 1) * 128],
                                    rhs=xt[:, 2 * j : 2 * j + 2,
                                           c * 512 : (c + 1) * 512],
                                    start=(j == 0),
                                    stop=(j == KP - 1),
                                    perf_mode=DR,
                                )
                        if b == 0 and g == 0 and m == 0:
                            # bias matmuls slot in after the first main
                            # matmul block, before the first tanh needs them
                            emit_bias()
                        th = thp.tile(
                            [128, GT], BF16, tag="tanh", name=f"th_{b}_{g}_{m}"
                        )
                        nc.scalar.activation(
                            out=th, in_=h_ps, func=TANH,
                            bias=bias_sb[:, m, b : b + 1],
                            scale=1.0 / (SW * SX),
                        )
                        ths[(g, m)] = th
                # column-packed v-dot: 4 chunks concurrently on col strips,
                # plus a K=1 matmul folding in the host-side mean-field
                # correction row (weight 1.0 at the same partition)
                for m in range(MT):
                    for ci in range(4):
                        gg, cc = divmod(ci, 2)
                        nc.tensor.matmul(
                            out=score_ps[32 * ci : 32 * ci + 1, :],
                            lhsT=v_sb[:, m : m + 1],
                            rhs=ths[(gg, m)][:, cc * 512 : (cc + 1) * 512],
                            start=(m == 0),
                            stop=False,
                            tile_position=(0, 32 * ci),
                        )
                for ci in range(4):
                    p0 = 4 * b + ci
                    nc.tensor.matmul(
                        out=score_ps[32 * ci : 32 * ci + 1, :],
                        lhsT=eye_sb[:, p0 : p0 + 1],
                        rhs=c_sb,
                        start=False,
                        stop=True,
                        tile_position=(0, 32 * ci),
                    )
                # exp fused into one full-height PSUM->SBUF copy; accum_out
                # yields the per-chunk softmax denominators for free
                stage = miscp.tile(
                    [128, 513], F32, tag="stage", bufs=2, name=f"stage_{b}"
                )
                nc.scalar.activation(
                    out=stage[:, 0:512], in_=score_ps, func=EXP,
                    accum_out=stage[:, 512:513],
                )
                # gather rows {0,32,64,96} -> escores[b] / esums[b]
                stager = stage.rearrange("(c r) f -> c r f", c=4)[:, 0, :]
                nc.gpsimd.dma_start(out=escomb[b : b + 1, :, :], in_=stager)

            # ---- softmax denominator + scale ----
            nc.vector.reduce_sum(
                out=esum, in_=escomb[:, :, 512:513], axis=mybir.AxisListType.XY
            )
            rs = miscp.tile([BL, 1], F32)
            nc.vector.reciprocal(out=rs, in_=esum)
            ob = miscp.tile([BL, S], F32)
            nc.vector.tensor_scalar_mul(
                out=ob.rearrange("p (c f) -> p c f", c=4),
                in0=escomb[:, :, 0:512], scalar1=rs,
            )
            # output DMA split across 4 queues
            for ci in range(4):
                first_eng[ci].dma_start(
                    out=out[:, ci * 512 : (ci + 1) * 512], in_=ob[:, ci * 512 : (ci + 1) * 512]
                )

    nc.compile()
    return nc


def kernel(static, dynamic, decoder_hidden, v, W):
    static = np.ascontiguousarray(np.asarray(static, dtype=np.float32))
    dynamic = np.ascontiguousarray(np.asarray(dynamic, dtype=np.float32))
    decoder_hidden = np.ascontiguousarray(np.asarray(decoder_hidden, dtype=np.float32))
    v = np.ascontiguousarray(np.asarray(v, dtype=np.float32))
    W = np.ascontiguousarray(np.asarray(W, dtype=np.float32))

    bf16 = ml_dtypes.bfloat16
    fp8 = ml_dtypes.float8_e4m3

    W12 = W[:, : 2 * H]                       # [768, 512]
    W3 = W[:, 2 * H :]                        # [768, 256]
    wt8 = (W12.T * SW).astype(fp8)            # [512, 768] fp8
    Wq = wt8.astype(np.float32).T / SW        # dequantized [768, 512]
    dW = W12 - Wq
    vv = v[0]
    # mean-field correction vectors (gamma folded in)
    a_vec = (GAMMA * (dW.T @ vv)).astype(np.float32)    # [512] . x~
    b_vec = (GAMMA * (Wq.T @ vv)).astype(np.float32)    # [512] . dx
    wt3_16 = np.ascontiguousarray(W3.T).astype(bf16)    # [256, 768]

    in_maps = []
    for c in range(NCORES):
        sl = slice(c * BL, (c + 1) * BL)
        x = np.concatenate(
            [static[sl].reshape(T, H), dynamic[sl].reshape(T, H)], axis=1
        )                                     # [T, 512] f32
        xq8 = (x * SX).astype(fp8)            # [T, 512] fp8
        xqf = xq8.astype(np.float32) / SX
        c_add = (xqf @ a_vec + (x - xqf) @ b_vec).astype(bf16)  # [T]
        x_t8 = np.ascontiguousarray(xq8.T)    # [512, T] fp8
        dec_t = np.ascontiguousarray(decoder_hidden[sl].T).astype(bf16)
        in_maps.append({
            "x_t": x_t8,
            "dec_t": dec_t,
            "wt": wt8,
            "wt3": wt3_16,
            "v": v,
            "corr": np.ascontiguousarray(c_add.reshape(BL, S)),
            "eye": np.eye(4 * BL, dtype=bf16),
        })

    if "nc" not in _CACHED:
        _CACHED["nc"] = build_bass()
    nc = _CACHED["nc"]

    trace = bool(int(os.environ.get("KERNEL_TRACE", "0")))
    res = run_bass_kernel_spmd(
        nc, in_maps, core_ids=list(range(NCORES)), trace=trace,
        trace_cores=list(range(NCORES)) if trace else None,
    )
    _CACHED["last_result"] = res

    out = np.concatenate([r["out"] for r in res.results], axis=0)
    return out


# revision 27
# speedup vs baseline: 1.0045x; 1.0045x over previous
"""Trainium2 Bass kernel for nn_Attention_27977416966318 (sparse_attention).

score[b,s] = v . tanh(W @ concat(static[b,s], dynamic[b,s], dec[b]))
out = softmax(score, axis=1)

Shapes: static/dynamic [64, 2048, 256] f32, decoder_hidden [64, 256],
v [1, 768], W [768, 768].  Output [64, 2048] f32.

Strategy: data-parallel over batch B=64 across 8 NeuronCores (8 batches
per core).  W @ cat = W12 @ [static;dynamic] + (W3 @ dec[b]), the last
term a per-batch bias computed once on-device in bf16.  The main GEMM
(contraction 512, outputs 768, 16384 tokens/core) runs in fp8-e4m3 with
perf_mode=DoubleRow (2 fp8 MACs/cell/cycle): W scaled x256, x scaled
x16, PSUM = 4096*h; the tanh activation applies scale=1/4096 + bias.

fp8 precision rescue: a first-order mean-field correction
  c[t] = gamma * [ (dW^T v) . x~_t + (W~^T v) . dx_t ],  gamma ~ E[tanh']
computed on the HOST (rank-2 dot per token, O(T*k)) and applied on-device
as exp(s+c) = exp(s)*exp(c) in the softmax tail (one DVE multiply).
This cuts max rel err from ~2.5e-2 (pure fp8) to ~1.26e-2 vs the 2e-2
gate.

The v-dot runs as 4 column-packed (tile_position) M=1 bf16 matmuls on
disjoint 32-col strips.  Each batch's exp is emitted one tanh-tile into
the next batch's scalar stream so the scalar engine never stalls on the
v-dot.  Softmax denominators come from a fused DVE reduce in the tail.
"""

import os

import numpy as np
import ml_dtypes

import concourse.bass as bass
from concourse import bacc
import concourse.mybir as mybir
import concourse.tile as tile
from concourse.bass_utils import run_bass_kernel_spmd

B, S, H = 64, 2048, 256
H3 = 3 * H          # 768
NCORES = 8
BL = B // NCORES    # 8 batches per core
T = BL * S          # 16384 tokens per core
KP = 2              # DoubleRow k-pair tiles (2 x 256 = 512 contraction)
MT = H3 // 128      # 6 output o-tiles
GT = 1024           # tokens per group (2 chunks of 512)
F32 = mybir.dt.float32
BF16 = mybir.dt.bfloat16
FP8 = mybir.dt.float8e4
DR = mybir.MatmulPerfMode.DoubleRow
TANH = mybir.ActivationFunctionType.Tanh
EXP = mybir.ActivationFunctionType.Exp

SW = 256.0          # W fp8 scale
SX = 16.0           # x fp8 scale
GAMMA = 0.5464      # E[tanh'(h)] mean-field coefficient

_CACHED = {}


def build_bass():
    nc = bacc.Bacc(None, target_bir_lowering=False, debug=False)
    x = nc.dram_tensor("x_t", [2 * H, T], FP8, kind="ExternalInput")
    xr = x.rearrange("(t p) n -> p t n", p=128)  # [128, 4, T]
    dec = nc.dram_tensor("dec_t", [H, BL], BF16, kind="ExternalInput")
    wt = nc.dram_tensor("wt", [2 * H, H3], FP8, kind="ExternalInput")
    wt3 = nc.dram_tensor("wt3", [H, H3], BF16, kind="ExternalInput")
    vv = nc.dram_tensor("v", [1, H3], F32, kind="ExternalInput")
    corr = nc.dram_tensor("corr", [BL, S], F32, kind="ExternalInput")
    out = nc.dram_tensor("out", [BL, S], F32, kind="ExternalOutput")

    with tile.TileContext(nc) as tc:
        with (
            tc.tile_pool(name="const", bufs=1) as constp,
            tc.tile_pool(name="xp", bufs=2) as xp,
            tc.tile_pool(name="thp", bufs=13) as thp,
            tc.tile_pool(name="misc", bufs=1) as miscp,
            tc.tile_pool(name="hps", bufs=3, space="PSUM") as hps,
            tc.tile_pool(name="sps", bufs=2, space="PSUM") as sps,
        ):
            # ---- PE warmup: zero matmuls so the HAM clock gate reaches
            # 8/8 and stays there by the time the real stream begins ----
            warm = constp.tile([128, 512], BF16)
            nc.vector.memset(warm, 0.0)
            warm_ps = sps.tile([128, 512], F32, tag="s", name="warm_ps")
            for i in range(16):
                nc.tensor.matmul(
                    out=warm_ps, lhsT=warm[:, 0:128], rhs=warm,
                    start=True, stop=True,
                )

            # ---- first two x tiles: spread k-tiles across the three
            # DMA-trigger queues so the first matmuls can start ASAP ----
            first_eng = [nc.sync, nc.scalar, nc.gpsimd, nc.sync]
            first_xt = xp.tile([128, 2 * KP, GT], FP8, tag="x", bufs=3, name="x_0_0")
            for kt_i in range(2 * KP):
                first_eng[kt_i].dma_start(
                    out=first_xt[:, kt_i, :], in_=xr[:, kt_i, 0:GT]
                )
            second_xt = xp.tile([128, 2 * KP, GT], FP8, tag="x", bufs=3, name="x_0_1")
            for kt_i in range(2 * KP):
                first_eng[kt_i].dma_start(
                    out=second_xt[:, kt_i, :], in_=xr[:, kt_i, GT : 2 * GT]
                )

            # ---- constants, in parallel with the x stream ----
            # wt is W12.T: [k, o] fp8 x256.  k-pair tiles (0,1) and (2,3).
            wtr = wt.rearrange("(t p) o -> p t o", p=128)
            wt_sb = constp.tile([128, 2 * KP, H3], FP8)
            nc.scalar.dma_start(out=wt_sb[:, 0:2, :], in_=wtr[:, 0:2, :])
            nc.scalar.dma_start(out=wt_sb[:, 2:4, :], in_=wtr[:, 2:4, :])
            # wt3 is W3.T: [k, o] bf16 for the decoder bias matmul
            wt3r = wt3.rearrange("(t p) o -> p t o", p=128)
            wt3_sb = constp.tile([128, 2, H3], BF16)
            nc.scalar.dma_start(out=wt3_sb, in_=wt3r)
            # v in bf16: the v-dot runs as a bf16 matmul (col-packable)
            v_sb = constp.tile([128, MT], BF16)
            nc.gpsimd.dma_start(out=v_sb, in_=vv[0].rearrange("(t p) -> p t", p=128))
            dec_sb = constp.tile([128, 2, BL], BF16)
            nc.gpsimd.dma_start(
                out=dec_sb, in_=dec.rearrange("(t p) b -> p t b", p=128)
            )
            # host-side mean-field correction, exp(c), [BL, 4, 512] f32
            ec_sb = constp.tile([BL, 4, 512], F32)
            nc.gpsimd.dma_start(out=ec_sb, in_=corr[:, :].rearrange(
                "b (c f) -> b c f", c=4))

            bias_sb = constp.tile([128, MT, BL], F32)

            def emit_bias():
                # bias[o, b] = sum_k W3T[k, o] dec[k, b]   (bf16)
                for m in range(MT):
                    bias_ps = sps.tile([128, BL], F32, tag="s", name=f"bias_ps_{m}")
                    for i in range(2):
                        nc.tensor.matmul(
                            out=bias_ps,
                            lhsT=wt3_sb[:, i, m * 128 : (m + 1) * 128],
                            rhs=dec_sb[:, i, :],
                            start=(i == 0),
                            stop=(i == 1),
                        )
                    nc.vector.tensor_copy(out=bias_sb[:, m, :], in_=bias_ps)

            # per b: 4 chunks x 512 exp-scores; esum: per-b softmax denom
            escomb = miscp.tile([BL, 4, 512], F32)
            esum = miscp.tile([BL, 1], F32)
            score_done = {}

            def emit_exp(b):
                # exp fused into one full-height PSUM->SBUF copy, then
                # gather rows {0,32,64,96} -> escomb[b]
                stage = miscp.tile(
                    [128, 512], F32, tag="stage", bufs=2, name=f"stage_{b}"
                )
                nc.scalar.activation(out=stage, in_=score_done[b], func=EXP)
                stager = stage.rearrange("(c r) f -> c r f", c=4)[:, 0, :]
                nc.gpsimd.dma_start(out=escomb[b : b + 1, :, :], in_=stager)

            # ---- main loop ----
            for b in range(BL):
                ths = {}
                # one PSUM bank holds the 4 chunk scores on partitions
                # 0/32/64/96 (column-group packing); the other rows are
                # zeroed (early, off the critical path) so one full-height
                # exp can read the whole bank
                score_ps = sps.tile([128, 512], F32, tag="s", name=f"sa_{b}")
                nc.vector.memset(score_ps, 0.0)
                for g in range(2):
                    tok0 = b * S + g * GT
                    if b == 0 and g == 0:
                        xt = first_xt
                    elif b == 0 and g == 1:
                        xt = second_xt
                    else:
                        xt = xp.tile(
                            [128, 2 * KP, GT], FP8, tag="x", bufs=3, name=f"x_{b}_{g}"
                        )
                        nc.sync.dma_start(
                            out=xt, in_=xr[:, :, tok0 : tok0 + GT]
                        )
                    for m in range(MT):
                        h_ps = hps.tile([128, GT], F32, tag="h", name=f"h_{b}_{g}_{m}")
                        for j in range(KP):
                            for c in range(2):
                                nc.tensor.matmul(
                                    out=h_ps[:, c * 512 : (c + 1) * 512],
                                    lhsT=wt_sb[:, 2 * j : 2 * j + 2,
                                               m * 128 : (m + 1) * 128],
                                    rhs=xt[:, 2 * j : 2 * j + 2,
                                           c * 512 : (c + 1) * 512],
                                    start=(j == 0),
                                    stop=(j == KP - 1),
                                    perf_mode=DR,
                                )
                        if b == 0 and g == 0 and m == 0:
                            # bias matmuls slot in after the first main
                            # matmul block, before the first tanh needs them
                            emit_bias()
                        th = thp.tile(
                            [128, GT], BF16, tag="tanh", name=f"th_{b}_{g}_{m}"
                        )
                        nc.scalar.activation(
                            out=th, in_=h_ps, func=TANH,
                            bias=bias_sb[:, m, b : b + 1],
                            scale=1.0 / (SW * SX),
                        )
                        ths[(g, m)] = th
                        if b > 0 and g == 0 and m == 0:
                            # previous batch's exp lands here in the scalar
                            # stream: its v-dot is long done, so no stall
                            emit_exp(b - 1)
                # column-packed v-dot: 4 chunks concurrently on col strips
                for m in range(MT):
                    for ci in range(4):
                        gg, cc = divmod(ci, 2)
                        nc.tensor.matmul(
                            out=score_ps[32 * ci : 32 * ci + 1, :],
                            lhsT=v_sb[:, m : m + 1],
                            rhs=ths[(gg, m)][:, cc * 512 : (cc + 1) * 512],
                            start=(m == 0),
                            stop=(m == MT - 1),
                            tile_position=(0, 32 * ci),
                        )
                score_done[b] = score_ps
            emit_exp(BL - 1)

            # ---- correction multiply + softmax denominator + scale ----
            # exp(s+c) = exp(s)*exp(c): one DVE multiply over [8, 2048]
            nc.vector.tensor_mul(out=escomb, in0=escomb, in1=ec_sb)
            nc.vector.reduce_sum(
                out=esum, in_=escomb, axis=mybir.AxisListType.XY
            )
            rs = miscp.tile([BL, 1], F32)
            nc.vector.reciprocal(out=rs, in_=esum)
            ob = miscp.tile([BL, S], F32)
            nc.vector.tensor_scalar_mul(
                out=ob.rearrange("p (c f) -> p c f", c=4),
                in0=escomb, scalar1=rs,
            )
            # output DMA split across the trigger queues
            for ci in range(4):
                first_eng[ci].dma_start(
                    out=out[:, ci * 512 : (ci + 1) * 512],
                    in_=ob[:, ci * 512 : (ci + 1) * 512],
                )

    nc.compile()
    return nc


def kernel(static, dynamic, decoder_hidden, v, W):
    static = np.ascontiguousarray(np.asarray(static, dtype=np.float32))
    dynamic = np.ascontiguousarray(np.asarray(dynamic, dtype=np.float32))
    decoder_hidden = np.ascontiguousarray(np.asarray(decoder_hidden, dtype=np.float32))
    v = np.ascontiguousarray(np.asarray(v, dtype=np.float32))
    W = np.ascontiguousarray(np.asarray(W, dtype=np.float32))

    bf16 = ml_dtypes.bfloat16
    fp8 = ml_dtypes.float8_e4m3

    W12 = W[:, : 2 * H]                       # [768, 512]
    W3 = W[:, 2 * H :]                        # [768, 256]
    wt8 = (W12.T * SW).astype(fp8)            # [512, 768] fp8
    Wq = wt8.astype(np.float32).T / SW        # dequantized [768, 512]
    dW = W12 - Wq
    vv = v[0]
    # mean-field correction vectors (gamma folded in)
    a_vec = (GAMMA * (dW.T @ vv)).astype(np.float32)    # [512] . x~
    b_vec = (GAMMA * (Wq.T @ vv)).astype(np.float32)    # [512] . dx
    wt3_16 = np.ascontiguousarray(W3.T).astype(bf16)    # [256, 768]

    in_maps = []
    for c in range(NCORES):
        sl = slice(c * BL, (c + 1) * BL)
        x = np.concatenate(
            [static[sl].reshape(T, H), dynamic[sl].reshape(T, H)], axis=1
        )                                     # [T, 512] f32
        xq8 = (x * SX).astype(fp8)            # [T, 512] fp8
        xqf = xq8.astype(np.float32) / SX
        ecorr = np.exp(
            (xqf @ a_vec + (x - xqf) @ b_vec).astype(np.float32)
        ).astype(np.float32)                  # [T] multiplicative factor
        x_t8 = np.ascontiguousarray(xq8.T)    # [512, T] fp8
        dec_t = np.ascontiguousarray(decoder_hidden[sl].T).astype(bf16)
        in_maps.append({
            "x_t": x_t8,
            "dec_t": dec_t,
            "wt": wt8,
            "wt3": wt3_16,
            "v": v,
            "corr": np.ascontiguousarray(ecorr.reshape(BL, S)),
        })

    if "nc" not in _CACHED:
        _CACHED["nc"] = build_bass()
    nc = _CACHED["nc"]

    trace = bool(int(os.environ.get("KERNEL_TRACE", "0")))
    res = run_bass_kernel_spmd(
        nc, in_maps, core_ids=list(range(NCORES)), trace=trace,
        trace_cores=list(range(NCORES)) if trace else None,
    )
    _CACHED["last_result"] = res

    out = np.concatenate([r["out"] for r in res.results], axis=0)
    return out
